# revision 35
# baseline (speedup 1.0000x reference)
"""AdaptiveTripletLoss distributed Trainium2 kernel (8 NeuronCores).

Strategy: shard by class. Host argsorts targets; each class becomes one
128-row padded block (max class count is ~105 for n=8192, C=100). 104
class slots = 13 blocks/core x 8 cores. Hardest-positive top-3 needs only
same-class distances, so each core computes 13 small 128x128 gram blocks
instead of a row-slab of the full 8192x8192 matrix. Class centers are
computed per-class locally and AllGathered. All floating-point loss math
runs on device; the host does data movement (sharding permutation) and the
final 8-way partial sum / count division.

Matmuls run in bf16 (selection ordering and center sums tolerate it; the
d_pos/d_neg value paths keep fp32 accumulation in PSUM). Top-3 one-hot is
built exactly with max8 + match_replace (no index arithmetic).
"""

import numpy as np
from concourse import bacc, mybir, tile, masks
from concourse.bass_types import AP
from concourse.bass_utils import run_bass_kernel_spmd

# Problem constants (hardcoded per harness contract)
N = 8192
D = 512
C = 100
NCORES = 8
BPC = 13              # class blocks per core
NSLOT = BPC * NCORES  # 104 class slots
P = 128               # rows per class block
KCH = D // P          # 4 contraction chunks
BIG = 1.0e4
EPS = 1.0e-12
SENT = 1.0e9          # match_replace sentinel (never present in negG)
REPL = 5.0            # match_replace imm (real negG values are <= ~1)
F32 = mybir.dt.float32
BF16 = mybir.dt.bfloat16

USE_REMOTE_DMA = False
_CACHED_NC = None


def _build_nc():
    nc = bacc.Bacc("TRN2", target_bir_lowering=False, debug=False,
                   num_devices=NCORES)
    emb_h = nc.declare_dram_parameter("emb", [P, BPC * D], BF16, isOutput=False)
    rw_h = nc.declare_dram_parameter("rwm", [P, BPC * BPC], BF16, isOutput=False)
    lw_h = nc.declare_dram_parameter("lw", [P, BPC], F32, isOutput=False)
    pb_h = nc.declare_dram_parameter("padbias", [1, BPC * P], BF16, isOutput=False)
    nb_h = nc.declare_dram_parameter("negbias", [NSLOT, BPC], F32, isOutput=False)
    ic_h = nc.declare_dram_parameter("invc", [BPC, 1], F32, isOutput=False)
    out_h = nc.declare_dram_parameter("out", [P, 1], F32, isOutput=True)

    AX = mybir.AxisListType
    OP = mybir.AluOpType
    AF = mybir.ActivationFunctionType

    with tile.TileContext(nc) as tc:
        with (
            tc.tile_pool(name="const", bufs=1) as cpool,
            tc.tile_pool(name="big", bufs=1) as bpool,
            tc.tile_pool(name="sm", bufs=1) as spool,
            tc.tile_pool(name="scr", bufs=4) as scr,
            tc.tile_pool(name="gt", bufs=5) as gt,
            tc.tile_pool(name="st", bufs=6) as st,
            tc.tile_pool(name="ps_t", bufs=3, space="PSUM") as ps_t,
            tc.tile_pool(name="ps_a", bufs=5, space="PSUM") as ps_a,
            tc.tile_pool(name="dram", bufs=1, space="DRAM") as dram,
        ):
            # ---- constants ----
            ident = cpool.tile([P, P], F32, tag="ident")
            masks.make_identity(nc, ident[:])
            ident_bf = cpool.tile([P, P], BF16, tag="ident_bf")
            masks.make_identity(nc, ident_bf[:])
            ones = cpool.tile([1, P], BF16, tag="ones")
            nc.vector.memset(ones[:], 1.0)
            ones_bcol = cpool.tile([P, 1], BF16, tag="ones_bcol")
            nc.vector.memset(ones_bcol[:], 1.0)

            # ---- persistent tiles ----
            Eraw = bpool.tile([P, BPC * D], BF16, tag="Eraw")
            Eb = bpool.tile([P, BPC * D], BF16, tag="Eb")
            ETb = bpool.tile([P, BPC * D], BF16, tag="ETb")
            rw_t = spool.tile([P, BPC * BPC], BF16, tag="rw")
            lw_t = spool.tile([P, BPC], F32, tag="lwt")
            pb_t = spool.tile([1, BPC * P], BF16, tag="pbt")
            nb_t = spool.tile([NSLOT, BPC], F32, tag="nbt")
            ic_t = spool.tile([BPC, 1], F32, tag="ict")
            ssq = spool.tile([P, BPC], F32, tag="ssq")
            nrm = spool.tile([P, BPC], F32, tag="nrm")
            rcp = spool.tile([P, BPC], F32, tag="rcp")
            tsc = spool.tile([P, BPC], F32, tag="tsc")
            a2 = spool.tile([P, BPC], F32, tag="a2")
            dpq = spool.tile([P, BPC], F32, tag="dpq")
            msc = spool.tile([P, BPC], F32, tag="msc")
            dnq = spool.tile([P, BPC], F32, tag="dnq")
            dsqs = spool.tile([P, 2 * BPC], F32, tag="dsqs")
            centers_l = spool.tile([BPC, D], BF16, tag="centers_l")
            centers_all = spool.tile([NSLOT, D], BF16, tag="centers_all")
            csq = spool.tile([NSLOT, D], F32, tag="csq")
            b2col = spool.tile([NSLOT, 1], F32, tag="b2col")
            comb = spool.tile([NSLOT, BPC], F32, tag="comb")
            ct2 = spool.tile([P, KCH * NSLOT], BF16, tag="ct2")

            # ---- input DMAs (issue spread across engine queues) ----
            qs = [0, 2, 5, 9, BPC]
            dma_engs = [nc.sync, nc.scalar, nc.sync, nc.scalar, nc.sync]
            for eng, (lo, hi) in zip(dma_engs, zip(qs[:-1], qs[1:])):
                eng.dma_start(out=Eraw[:, lo * D:hi * D],
                              in_=emb_h[:, lo * D:hi * D])
            nc.sync.dma_start(out=rw_t[:], in_=rw_h[:])
            nc.sync.dma_start(out=lw_t[:], in_=lw_h[:])
            nc.sync.dma_start(out=pb_t[:], in_=pb_h[:])
            nc.sync.dma_start(out=nb_t[:], in_=nb_h[:])
            nc.sync.dma_start(out=ic_t[:], in_=ic_h[:])

            # ---- W1: row sum-of-squares wave ----
            for b in range(BPC):
                bsl = slice(b * D, (b + 1) * D)
                sq = scr.tile([P, D], F32, tag="sq")
                if b % 3 != 1:
                    nc.vector.scalar_tensor_tensor(
                        sq[:], in0=Eraw[:, bsl], scalar=1.0, in1=Eraw[:, bsl],
                        op0=OP.mult, op1=OP.mult, accum_out=ssq[:, b:b + 1])
                else:
                    nc.scalar.activation(sq[:], Eraw[:, bsl], AF.Square,
                                         accum_out=ssq[:, b:b + 1])
            # ---- W2: batched norm scalars (joins aligned to DMA quarters) ----
            for lo, hi in ((0, 2), (2, 5), (5, 9), (9, BPC)):
                nc.scalar.activation(nrm[:, lo:hi], ssq[:, lo:hi], AF.Sqrt)
                nc.vector.tensor_scalar_max(nrm[:, lo:hi], nrm[:, lo:hi], EPS)
                nc.vector.reciprocal(rcp[:, lo:hi], nrm[:, lo:hi])
            nc.vector.tensor_mul(tsc[:], rcp[:], rcp[:])
            nc.vector.tensor_mul(a2[:], ssq[:], tsc[:])
            # ---- W3: scale to unit rows (bf16), alternating engines ----
            for b in range(BPC):
                bsl = slice(b * D, (b + 1) * D)
                if b % 3 == 1:
                    nc.scalar.activation(Eb[:, bsl], Eraw[:, bsl], AF.Copy,
                                         scale=rcp[:, b:b + 1])
                else:
                    nc.vector.tensor_scalar(Eb[:, bsl], Eraw[:, bsl],
                                            rcp[:, b:b + 1], None, op0=OP.mult)
            # ---- W4: class centers + AllGather ----
            pcn = ps_a.tile([BPC, D], F32, tag="pa")
            for b in range(BPC):
                nc.tensor.matmul(pcn[:], lhsT=rw_t[:, b * BPC:(b + 1) * BPC],
                                 rhs=Eb[:, b * D:(b + 1) * D],
                                 start=(b == 0), stop=(b == BPC - 1))
            nc.vector.tensor_scalar_mul(centers_l[:], pcn[:], ic_t[:])
            if USE_REMOTE_DMA:
                # Flatten [13,512] -> [128,52]; broadcast region r to peer
                # (tpb ^ r); receiver region r holds sender (self ^ r)'s
                # centers (host compensates the order in negbias).
                FW = BPC * D // P  # 52
                cl128 = spool.tile([P, FW], F32, tag="cl128")
                call_all = spool.tile([P, NCORES * FW], F32, tag="call_all")
                t6 = dram.tile([BPC, D], F32, tag="t6")
                t7 = dram.tile([NCORES, BPC, D], F32, tag="t7")
                with (
                    nc.semaphore("rdb_rsem") as rsem,
                    nc.semaphore("rdb_lsem") as lsem,
                    nc.semaphore("rdb_dsem") as dsem,
                ):
                    # [13,512] -> DRAM -> [128,52] flat view
                    nc.sync.dma_start(out=t6[:], in_=centers_l[:])
                    t6v = t6[:]
                    t6flat = AP(t6v.tensor, t6v.offset, [[FW, P], [1, FW]])
                    nc.sync.dma_start(out=cl128[:], in_=t6flat)
                    with tc.tile_critical():
                        for r in range(NCORES):
                            rdests = [None] * NCORES
                            rdests[r] = (0, r)
                            nc.gpsimd.remote_dma_broadcast(
                                out_ap=call_all[:, r * FW:(r + 1) * FW],
                                in_ap=cl128[:],
                                remote_sem=rsem,
                                local_sem=lsem,
                                rdests=rdests,
                            )
                        nc.gpsimd.trigger_dma(count=None)
                        nc.gpsimd.wait_ge(rsem, 16)
                        # unscramble: [128, 8, 52] -> DRAM (r, p, e) -> [104, 512]
                        ca = call_all[:]
                        ca3 = AP(ca.tensor, ca.offset,
                                 [ca.ap[0], [FW, NCORES], [1, FW]])
                        t7v = t7[:]
                        t7o = AP(t7v.tensor, t7v.offset,
                                 [[FW, P], [BPC * D, NCORES], [1, FW]])
                        nc.gpsimd.dma_start(out=t7o, in_=ca3).then_inc(dsem, 16)
                        nc.gpsimd.wait_ge(dsem, 16)
                        t7i = AP(t7v.tensor, t7v.offset, [[D, NSLOT], [1, D]])
                        nc.gpsimd.dma_start(out=centers_all[:], in_=t7i).then_inc(dsem, 16)
                        nc.gpsimd.wait_ge(dsem, 32)
            else:
                cc_in = dram.tile([BPC, D], BF16, tag="cc_in")
                cc_out = dram.tile([NSLOT, D], BF16, addr_space="Shared",
                                   tag="cc_out")
                nc.sync.dma_start(out=cc_in[:], in_=centers_l[:])
                nc.gpsimd.collective_compute(
                    "AllGather", OP.bypass,
                    replica_groups=[list(range(NCORES))],
                    ins=[cc_in[:].opt()],
                    outs=[cc_out[:].opt()],
                )
                nc.sync.dma_start(out=centers_all[:], in_=cc_out[:])


            # ---- W5: transpose wave ----
            for b in range(BPC):
                pt = ps_t.tile([P, D], BF16, tag="pt")
                for k in range(KCH):
                    nc.tensor.transpose(pt[:, k * P:(k + 1) * P],
                                        Eb[:, b * D + k * P:b * D + (k + 1) * P],
                                        ident_bf[:])
                if b % 2 == 0:
                    nc.vector.tensor_copy(ETb[:, b * D:(b + 1) * D], pt[:])
                else:
                    nc.scalar.activation(ETb[:, b * D:(b + 1) * D], pt[:], AF.Copy)

            # ---- W6: gram wave + negate ----
            negGs = []
            for b in range(BPC):
                pg = ps_a.tile([P, P], F32, tag="pa")
                for k in range(KCH):
                    sl = slice(b * D + k * P, b * D + (k + 1) * P)
                    nc.tensor.matmul(pg[:], lhsT=ETb[:, sl], rhs=ETb[:, sl],
                                     start=(k == 0), stop=False)
                nc.tensor.matmul(pg[:], lhsT=ones[:],
                                 rhs=pb_t[0:1, b * P:(b + 1) * P],
                                 start=False, stop=True)
                negG = gt.tile([P, P], F32, tag="negG")
                if b % 2 == 0:
                    nc.vector.tensor_scalar_mul(negG[:], pg[:], -1.0)
                else:
                    nc.scalar.activation(negG[:], pg[:], AF.Copy, scale=-1.0)
                negGs.append(negG)

            # ---- W7: top-3 selection wave (DVE) ----
            v8all = spool.tile([P, 8 * BPC], F32, tag="v8all")
            Sbs = []
            for b in range(BPC):
                negG = negGs[b]
                v8 = v8all[:, 8 * b:8 * b + 8]
                nc.vector.max(v8, negG[:])
                nc.vector.memset(v8all[:, 8 * b + 3:8 * b + 8], SENT)
                Gm = gt.tile([P, P], F32, tag="Gm")
                nc.vector.match_replace(Gm[:], v8, negG[:], REPL)
                Sb = gt.tile([P, P], BF16, tag="Sb")
                nc.vector.tensor_scalar(Sb[:], Gm[:], REPL - 1.0, None, op0=OP.is_ge)
                Sbs.append(Sb)

            # ---- W8: S transpose + pos-center matmul + |3pc|^2 wave ----
            p2 = spool.tile([P, BPC], F32, tag="p2")
            for b in range(BPC):
                pst = ps_t.tile([P, P], BF16, tag="pt")
                nc.tensor.transpose(pst[:], Sbs[b][:], ident_bf[:])
                S_T = gt.tile([P, P], BF16, tag="S_T")
                if b % 2 == 0:
                    nc.vector.tensor_copy(S_T[:], pst[:])
                else:
                    nc.scalar.activation(S_T[:], pst[:], AF.Copy)
                ppc = ps_a.tile([P, D], F32, tag="pa")
                nc.tensor.matmul(ppc[:], lhsT=S_T[:], rhs=Eb[:, b * D:(b + 1) * D],
                                 start=True, stop=True)
                sq2 = scr.tile([P, D], F32, tag="sq")
                nc.scalar.activation(sq2[:], ppc[:], AF.Square,
                                     accum_out=p2[:, b:b + 1])

            # ---- W9: d_pos^2 = a2 + p2/9 + (2/3)*sum(top3 of -G) ----
            v3s = spool.tile([P, BPC], F32, tag="v3s")
            v8_3 = AP(v8all[:].tensor, v8all[:].offset,
                      [v8all[:].ap[0], [8, BPC], [1, 3]])
            nc.vector.tensor_reduce(v3s[:], v8_3, axis=AX.X, op=OP.add)
            t2a = spool.tile([P, BPC], F32, tag="t2a")
            nc.vector.tensor_scalar(t2a[:], p2[:], 1.0 / 9.0, None, op0=OP.mult)
            nc.vector.scalar_tensor_tensor(dpq[:], in0=v3s[:], scalar=2.0 / 3.0,
                                           in1=t2a[:], op0=OP.mult, op1=OP.add)
            nc.vector.tensor_tensor(dsqs[:, 0:BPC], dpq[:], a2[:], op=OP.add)

            # ---- pos-branch sqrt early (off the post-collective tail) ----
            dsc = spool.tile([P, 2 * BPC], F32, tag="dsc")
            ds = spool.tile([P, 2 * BPC], F32, tag="ds")
            nc.vector.tensor_scalar_max(dsc[:, 0:BPC], dsqs[:, 0:BPC], EPS)
            nc.scalar.activation(ds[:, 0:BPC], dsc[:, 0:BPC], AF.Sqrt)

            # ---- phase 5: centers prep (force-scheduled after local work) ----
            tc.tile_set_cur_wait(5.0)
            nc.vector.scalar_tensor_tensor(csq[:], in0=centers_all[:],
                                           scalar=1.0, in1=centers_all[:],
                                           op0=OP.mult, op1=OP.mult,
                                           accum_out=b2col[:])
            ptc = ps_t.tile([P, KCH * NSLOT], BF16, tag="pt")
            for k in range(KCH):
                nc.tensor.transpose(ptc[:, k * NSLOT:(k + 1) * NSLOT],
                                    centers_all[:, k * P:(k + 1) * P],
                                    ident_bf[0:NSLOT, 0:NSLOT])
                nc.vector.tensor_scalar_mul(ct2[:, k * NSLOT:(k + 1) * NSLOT],
                                            ptc[:, k * NSLOT:(k + 1) * NSLOT],
                                            -2.0)
            # comb_c[c, b] = b2[c] + negbias[c, b]  (free-stride-0 broadcast)
            b2b = AP(b2col[:].tensor, b2col[:].offset,
                     [b2col[:].ap[0], [0, BPC]])
            nc.vector.tensor_tensor(comb[:], nb_t[:], b2b, op=OP.add)

            # ---- phase 6: fat transposed score matmuls; PE re-transpose + DVE min ----
            xs = spool.tile([P, BPC], F32, tag="xs")
            groups = [(0, 4), (4, 8), (8, 12), (12, BPC)]
            pq = []
            for _gi in range(len(groups)):
                pq_t = ps_a.tile([NSLOT, 4 * P], F32, tag="pa")
                pq.append(pq_t)
            for gi, (lo, hi) in enumerate(groups):
                nb = hi - lo
                for k in range(KCH):
                    base = ETb[:, lo * D + k * P:lo * D + (k + 1) * P]
                    rhs3 = AP(base.tensor, base.offset,
                              [base.ap[0], [D, nb], [1, P]])
                    nc.tensor.matmul(pq[gi][:, 0:nb * P],
                                     lhsT=ct2[:, k * NSLOT:(k + 1) * NSLOT],
                                     rhs=rhs3,
                                     start=(k == 0), stop=(k == KCH - 1))
            for gi, (lo, hi) in enumerate(groups):
                for j in range(hi - lo):
                    ssb = gt.tile([NSLOT, P], F32, tag="ssb")
                    nc.vector.tensor_scalar(
                        ssb[:], pq[gi][:, j * P:(j + 1) * P],
                        1.0, comb[:, lo + j:lo + j + 1],
                        op0=OP.mult, op1=OP.add)
                    pts = ps_t.tile([P, NSLOT], F32, tag="pt")
                    nc.tensor.transpose(pts[:], ssb[:],
                                        ident[0:NSLOT, 0:NSLOT])
                    nc.vector.tensor_reduce(msc[:, lo + j:lo + j + 1], pts[:],
                                            axis=AX.X, op=OP.min)
                nc.vector.tensor_tensor(dsqs[:, BPC + lo:BPC + hi],
                                        msc[:, lo:hi], a2[:, lo:hi], op=OP.add)
                nc.vector.tensor_scalar_max(dsc[:, BPC + lo:BPC + hi],
                                            dsqs[:, BPC + lo:BPC + hi], EPS)
                nc.scalar.activation(ds[:, BPC + lo:BPC + hi],
                                     dsc[:, BPC + lo:BPC + hi], AF.Sqrt)
                nc.vector.tensor_sub(xs[:, lo:hi], ds[:, lo:hi],
                                     ds[:, BPC + lo:BPC + hi])

            # ---- phase 7: loss tail ----
            # softplus(x) = 0.5x + h(x^2), deg-5 poly in u=x^2 (|err|<5e-6 on |x|<2.6)
            PC = [6.931485008076e-01, 1.249840895147e-01, -5.177011703000e-03,
                  3.240810187699e-04, -1.812813478166e-05, 5.616111839003e-07]
            uq = spool.tile([P, BPC], F32, tag="uq")
            nc.vector.tensor_mul(uq[:], xs[:], xs[:])
            ph = spool.tile([P, BPC], F32, tag="ph")
            nc.vector.tensor_scalar_mul(ph[:], uq[:], PC[5])
            for k in (4, 3, 2, 1):
                nc.vector.scalar_tensor_tensor(ph[:], in0=ph[:], scalar=PC[k],
                                               in1=uq[:], op0=OP.add,
                                               op1=OP.mult)
            lp = spool.tile([P, BPC], F32, tag="lp")
            nc.vector.scalar_tensor_tensor(lp[:], in0=xs[:], scalar=0.5,
                                           in1=ph[:], op0=OP.mult, op1=OP.add)
            nc.vector.tensor_scalar_add(lp[:], lp[:], PC[0])
            wl = spool.tile([P, BPC], F32, tag="wl")
            accrow = spool.tile([P, 1], F32, tag="accrow")
            nc.vector.scalar_tensor_tensor(wl[:], in0=lp[:], scalar=1.0,
                                           in1=lw_t[:], op0=OP.mult,
                                           op1=OP.mult, accum_out=accrow[:])
            nc.sync.dma_start(out=out_h[:], in_=accrow[:])

    nc.finalize()
    return nc


def _get_nc():
    global _CACHED_NC
    if _CACHED_NC is None:
        _CACHED_NC = _build_nc()
    return _CACHED_NC


def _prep_inputs(embeddings, targets):
    """Host-side sharding: class-sorted, padded to 128-row class blocks."""
    import ml_dtypes
    emb = np.ascontiguousarray(np.asarray(embeddings, dtype=np.float32))
    tgt = np.asarray(targets).astype(np.int64)
    counts = np.bincount(tgt, minlength=C)
    if counts.max() > P:
        raise ValueError(f"class count {counts.max()} exceeds block size {P}")
    order = np.argsort(tgt, kind="stable")
    offs = np.zeros(C + 1, dtype=np.int64)
    np.cumsum(counts, out=offs[1:])

    emb_pad = np.zeros((NCORES, P, BPC * D), dtype=ml_dtypes.bfloat16)
    rwm = np.zeros((NCORES, P, BPC * BPC), dtype=ml_dtypes.bfloat16)
    lw = np.zeros((NCORES, P, BPC), dtype=np.float32)
    padbias = np.zeros((NCORES, 1, BPC * P), dtype=ml_dtypes.bfloat16)
    negbias = np.zeros((NCORES, NSLOT, BPC), dtype=np.float32)
    invc = np.zeros((NCORES, BPC, 1), dtype=np.float32)

    for slot in range(NSLOT):
        core, b = slot // BPC, slot % BPC
        if slot < C:
            cnt = int(counts[slot])
            rows = order[offs[slot]:offs[slot] + cnt]
            emb_pad[core, :cnt, b * D:(b + 1) * D] = emb[rows]
        else:
            cnt = 0
        padbias[core, 0, b * P + cnt:(b + 1) * P] = BIG
        if cnt:
            rwm[core, :cnt, b * BPC + b] = 1.0
            invc[core, b, 0] = 1.0 / cnt
            if cnt >= 2:
                lw[core, :cnt, b] = 1.0
        # exclude own class and empty/pad class slots from the negative min
        for c in range(NSLOT):
            if c == slot or c >= C or counts[c] == 0:
                negbias[core, c, b] = BIG

    denom = float(counts[counts >= 2].sum())
    return emb_pad, rwm, lw, padbias, negbias, invc, denom


def _make_in_maps(emb_pad, rwm, lw, padbias, negbias, invc):
    return [
        {
            "emb": emb_pad[i],
            "rwm": rwm[i],
            "lw": lw[i],
            "padbias": padbias[i],
            "negbias": negbias[i],
            "invc": invc[i],
        }
        for i in range(NCORES)
    ]


def kernel(embeddings, targets, num_classes):
    import time
    emb_pad, rwm, lw, padbias, negbias, invc, denom = _prep_inputs(
        embeddings, targets)
    nc = _get_nc()
    in_maps = _make_in_maps(emb_pad, rwm, lw, padbias, negbias, invc)
    res = None
    for attempt in range(3):
        try:
            res = run_bass_kernel_spmd(nc, in_maps, core_ids=list(range(NCORES)))
            break
        except Exception:
            # transient device wedges (NRT_EXEC_UNIT_UNRECOVERABLE) clear
            # after a cooldown; retry rather than failing the whole call
            if attempt == 2:
                raise
            time.sleep(45)
    parts = [np.asarray(res.results[i]["out"], dtype=np.float64).sum() for i in range(NCORES)]
    loss = np.float32(np.sum(np.asarray(parts, dtype=np.float64)) / max(denom, 1.0))
    return np.asarray(loss, dtype=np.float32)
